# revision 9
# baseline (speedup 1.0000x reference)
"""Sparse (masked) multi-head attention on 8 Trainium2 NeuronCores.

Problem: nodes [2,2048,512], edge_mask [2,2048,2048] (bool),
q/kv/o linear layers with H=8 heads of DH=64.

Sharding: batch x head-group.  Core c handles batch b = c//4 and head group
g = c%4 (heads 2g, 2g+1 = inner columns g*128:(g+1)*128).  Each core
computes its two heads' attention over the full sequence plus its partial
contribution to the output projection; the host sums the 4 partials per
batch and adds bo.

Per-core dataflow (all matmuls bf16 inputs, fp32 PSUM accumulation):
  qT/kT [dh=128, N]  = wq_sliceT @ nodesT (+bias)        (dh on partitions)
  v     [N, dh=128]  = nodesT.T @ wv_slice (+bias via ones-row matmul)
  per head h: simT[j,i] = kT_h.T @ qT_h                  (j on partitions)
              PT = exp(simT * DH**-0.5)   (ScalarE, free scale, bf16 out)
              PT *= maskT                  (VectorE, bf16 2x mode)
              numT[0:64,i] / den[64,i] = [v_h | 1].T @ PT  (ones col -> denom)
              attnT_h = numT * recip(den)  (recip + DMA partition-broadcast)
  out[i,:] += attnT.T @ wo_slice           (contraction over both heads)
"""
import numpy as np
import ml_dtypes

import concourse.bass as bass
import concourse.bacc as bacc
import concourse.tile as tile
from concourse import mybir
from concourse.bass_utils import run_bass_kernel_spmd

B, N, DIM = 2, 2048, 512
H, DH = 8, 64
INNER = H * DH
SCALE = DH ** -0.5
NCORES = 8
HEADS_PER_CORE = 2
HG = 128            # inner columns per core (2 heads x 64)
NJB = N // 128      # 16 j-blocks
NISL = N // 512     # 4 i-slices of 512
NC_DIM = DIM // 128  # 4 contraction chunks over DIM

BF16 = mybir.dt.bfloat16
F32 = mybir.dt.float32
ts = bass.ts


def _build():
    nc = bacc.Bacc(monotonic_sem_count=0)
    nT_d = nc.declare_dram_parameter("nodesT", [DIM, N], BF16, isOutput=False)
    maskT_d = nc.declare_dram_parameter("maskT", [N, N], BF16, isOutput=False)
    wq_d = nc.declare_dram_parameter("wq_s", [DIM, HG], BF16, isOutput=False)
    wk_d = nc.declare_dram_parameter("wk_s", [DIM, HG], BF16, isOutput=False)
    wv_d = nc.declare_dram_parameter("wv_s", [DIM, HG], BF16, isOutput=False)
    wo_d = nc.declare_dram_parameter("wo_s", [HG, DIM], BF16, isOutput=False)
    bq_d = nc.declare_dram_parameter("bq_s", [HG, 1], F32, isOutput=False)
    bk_d = nc.declare_dram_parameter("bk_s", [HG, 1], F32, isOutput=False)
    bv_d = nc.declare_dram_parameter("bv_s", [1, HG], BF16, isOutput=False)
    out_d = nc.declare_dram_parameter("out", [N, DIM], F32, isOutput=True)

    with tile.TileContext(nc) as tc:
        with (
            tc.tile_pool(name="persist", bufs=1) as persist,
            tc.tile_pool(name="ptp", bufs=3) as ptp,
            tc.tile_pool(name="denp", bufs=2) as denp,
            tc.tile_pool(name="outp", bufs=3) as outp,
            # PSUM is 8 banks total; two 4-bank pools, one slot each, shared
            # across phases: psA = {q/k proj, sim, even o-proj}, psB = {v proj,
            # num, odd o-proj}.
            tc.tile_pool(name="psA", bufs=1, space="PSUM") as psA,
            tc.tile_pool(name="psB", bufs=1, space="PSUM") as psB,
        ):
            # ---- loads ----
            nT = persist.tile([128, NC_DIM, N], BF16)
            nc.sync.dma_start(
                out=nT[:], in_=nT_d.rearrange("(c p) n -> p c n", p=128)
            )
            wq = persist.tile([128, NC_DIM, HG], BF16)
            nc.sync.dma_start(
                out=wq[:], in_=wq_d.rearrange("(c p) m -> p c m", p=128)
            )
            wk = persist.tile([128, NC_DIM, HG], BF16)
            nc.sync.dma_start(
                out=wk[:], in_=wk_d.rearrange("(c p) m -> p c m", p=128)
            )
            wv = persist.tile([128, NC_DIM, HG], BF16)
            nc.sync.dma_start(
                out=wv[:], in_=wv_d.rearrange("(c p) m -> p c m", p=128)
            )
            wo = persist.tile([HG, DIM], BF16)
            nc.sync.dma_start(out=wo[:], in_=wo_d[:])
            bq = persist.tile([HG, 1], F32)
            nc.sync.dma_start(out=bq[:], in_=bq_d[:])
            bk = persist.tile([HG, 1], F32)
            nc.sync.dma_start(out=bk[:], in_=bk_d[:])
            bv = persist.tile([1, HG], BF16)
            nc.sync.dma_start(out=bv[:], in_=bv_d[:])
            ones = persist.tile([1, 128], BF16)
            nc.vector.memset(ones[:], 1.0)

            maskT = persist.tile([128, NJB, N], BF16)
            for jb in range(NJB):
                nc.sync.dma_start(
                    out=maskT[:, jb, :], in_=maskT_d[ts(jb, 128), :]
                )

            # ---- projections ----
            qT = persist.tile([128, N], BF16)
            kT = persist.tile([128, N], BF16)
            for dst, w, b in ((qT, wq, bq), (kT, wk, bk)):
                for half in range(2):
                    pps = psA.tile([128, N // 2], F32, tag=f"sim{half}")
                    for isl in range(2):
                        for c in range(NC_DIM):
                            nc.tensor.matmul(
                                pps[:, ts(isl, 512)],
                                lhsT=w[:, c, :],
                                rhs=nT[:, c, ts(half * 2 + isl, 512)],
                                start=(c == 0),
                                stop=(c == NC_DIM - 1),
                            )
                    nc.vector.tensor_scalar_add(
                        dst[:, ts(half, N // 2)], pps[:], b[:]
                    )

            # v rows [j, dh] with a ones column appended per head:
            # cols 0:64 = head0 v, col 64 = 1, cols 65:129 = head1 v, col 129 = 1
            v_sb = persist.tile([128, NJB, 130], BF16)
            for jb in range(NJB):
                vps = psB.tile([128, HG], F32, tag="B")
                for c in range(NC_DIM):
                    nc.tensor.matmul(
                        vps[:],
                        lhsT=nT[:, c, ts(jb, 128)],
                        rhs=wv[:, c, :],
                        start=(c == 0),
                        stop=False,
                    )
                nc.tensor.matmul(
                    vps[:], lhsT=ones[:, 0:HG], rhs=bv[:], start=False, stop=True
                )
                nc.vector.tensor_copy(v_sb[:, jb, 0:64], vps[:, 0:64])
                nc.vector.tensor_copy(v_sb[:, jb, 65:129], vps[:, 64:128])
            nc.vector.memset(v_sb[:, :, 64:65], 1.0)
            nc.vector.memset(v_sb[:, :, 129:130], 1.0)

            # ---- attention per head ----
            attnT = persist.tile([128, N], BF16)
            for h in range(HEADS_PER_CORE):
                hp = ts(h, 64)  # partitions of this head's dh in qT/kT
                nps = psB.tile([65, N], F32, tag="B")
                for jb in range(NJB):
                    pt = ptp.tile([128, N], BF16, tag="pt")
                    # two 2-bank sim slots: ACT exps one half while PE
                    # fills the other -> ScalarE streams continuously
                    for half in range(2):
                        sps = psA.tile([128, N // 2], F32, tag=f"sim{half}")
                        for isl in range(2):
                            nc.tensor.matmul(
                                sps[:, ts(isl, 512)],
                                lhsT=kT[hp, ts(jb, 128)],
                                rhs=qT[hp, ts(half * 2 + isl, 512)],
                                start=True,
                                stop=True,
                            )
                        nc.scalar.activation(
                            out=pt[:, ts(half, N // 2)],
                            in_=sps[:],
                            func=mybir.ActivationFunctionType.Exp,
                            scale=SCALE,
                        )
                    nc.vector.tensor_mul(pt[:], pt[:], maskT[:, jb, :])
                    for isl in range(NISL):
                        nc.tensor.matmul(
                            nps[:, ts(isl, 512)],
                            lhsT=v_sb[:, jb, ts(h, 65)],
                            rhs=pt[:, ts(isl, 512)],
                            start=(jb == 0),
                            stop=(jb == NJB - 1),
                        )
                # attnT_h = numT * (1/den), den broadcast across partitions
                # reciprocal_approx_fast (custom DVE op) misreads sources at
                # base_partition 64 — copy the denominator row to partition 0
                # first.
                den1 = denp.tile([1, N], F32, tag="den1")
                nc.vector.tensor_copy(den1[:], nps[64:65, :])
                rec1 = denp.tile([1, N], F32, tag="rec1")
                nc.vector.reciprocal_approx_fast(out=rec1[:], in_=den1[:])
                rec = denp.tile([64, N], F32, tag="rec")
                nc.gpsimd.partition_broadcast(rec[:], rec1[:])
                nc.vector.tensor_mul(attnT[hp, :], nps[0:64, :], rec[:])

            # ---- output projection (contraction over both heads' inner dim) ----
            for ib in range(NJB):
                ops = psA.tile([128, DIM], F32, tag=f"sim{ib % 2}")
                nc.tensor.matmul(
                    ops[:], lhsT=attnT[:, ts(ib, 128)], rhs=wo[:],
                    start=True, stop=True,
                )
                osb = outp.tile([128, DIM], F32, tag="osb")
                nc.vector.tensor_copy(osb[:], ops[:])
                nc.sync.dma_start(out=out_d[ts(ib, 128), :], in_=osb[:])

    # Bacc.compile runs generate_event_semaphores, which splits multi-sem
    # waits down to the 1-wait-per-instruction limit this walrus enforces.
    nc.compile()

    # Bacc's dce_regs leaves the (unread) engine-preamble register writes
    # behind at this kernel size, with deferred reg_id=-1 — walrus then
    # fails "Reg has not been allocated yet".  Nothing reads them, so any
    # valid unique per-engine id works.
    from collections import defaultdict

    next_id = defaultdict(lambda: 8)
    for a in nc.m.functions[0].allocations:
        if type(a).__name__ == "Register" and a.reg_id == -1:
            a.reg_id = next_id[str(a.engine)]
            next_id[str(a.engine)] += 1
    return nc


_NC_CACHE = None


def _get_nc():
    global _NC_CACHE
    if _NC_CACHE is None:
        _NC_CACHE = _build()
    return _NC_CACHE


def _prep_in_maps(nodes, edge_mask, wq, bq, wkv, bkv, wo, bo):
    bf16 = ml_dtypes.bfloat16
    wk_full, wv_full = wkv[:, :INNER], wkv[:, INNER:]
    bk_full, bv_full = bkv[:INNER], bkv[INNER:]
    per_batch = []
    for b in range(B):
        per_batch.append(
            (
                np.ascontiguousarray(nodes[b].T).astype(bf16),
                np.ascontiguousarray(edge_mask[b].T).astype(bf16),
            )
        )
    in_maps = []
    for core in range(NCORES):
        b, g = core // 4, core % 4
        cs = slice(g * HG, (g + 1) * HG)
        nT_b, maskT_b = per_batch[b]
        in_maps.append(
            {
                "nodesT": nT_b,
                "maskT": maskT_b,
                "wq_s": np.ascontiguousarray(wq[:, cs]).astype(bf16),
                "wk_s": np.ascontiguousarray(wk_full[:, cs]).astype(bf16),
                "wv_s": np.ascontiguousarray(wv_full[:, cs]).astype(bf16),
                "wo_s": np.ascontiguousarray(wo[cs, :]).astype(bf16),
                "bq_s": np.ascontiguousarray(bq[cs]).reshape(HG, 1).astype(np.float32),
                "bk_s": np.ascontiguousarray(bk_full[cs]).reshape(HG, 1).astype(np.float32),
                "bv_s": np.ascontiguousarray(bv_full[cs]).reshape(1, HG).astype(bf16),
            }
        )
    return in_maps


def kernel(nodes, edge_mask, wq, bq, wkv, bkv, wo, bo, _trace=False, _trace_kwargs=None):
    nodes = np.asarray(nodes, dtype=np.float32)
    edge_mask = np.asarray(edge_mask)
    wq = np.asarray(wq, dtype=np.float32)
    bq = np.asarray(bq, dtype=np.float32)
    wkv = np.asarray(wkv, dtype=np.float32)
    bkv = np.asarray(bkv, dtype=np.float32)
    wo = np.asarray(wo, dtype=np.float32)
    bo = np.asarray(bo, dtype=np.float32)

    nc = _get_nc()
    in_maps = _prep_in_maps(nodes, edge_mask, wq, bq, wkv, bkv, wo, bo)
    kw = {}
    if _trace:
        kw = dict(trace=True, **(_trace_kwargs or {}))
    res = run_bass_kernel_spmd(nc, in_maps, list(range(NCORES)), **kw)
    out = np.zeros((B, N, DIM), np.float32)
    for core in range(NCORES):
        out[core // 4] += res.results[core]["out"]
    out += bo[None, None, :]
    if _trace:
        return out, res
    return out


# revision 11
# speedup vs baseline: 1.0991x; 1.0991x over previous
"""Sparse (masked) multi-head attention on 8 Trainium2 NeuronCores.

Problem: nodes [2,2048,512], edge_mask [2,2048,2048] (bool),
q/kv/o linear layers with H=8 heads of DH=64.

Sharding: batch x head-group.  Core c handles batch b = c//4 and head group
g = c%4 (heads 2g, 2g+1 = inner columns g*128:(g+1)*128).  Each core
computes its two heads' attention over the full sequence plus its partial
contribution to the output projection; the host sums the 4 partials per
batch and adds bo.

Per-core dataflow (all matmuls bf16 inputs, fp32 PSUM accumulation):
  qT/kT [dh=128, N]  = wq_sliceT @ nodesT (+bias)        (dh on partitions)
  v     [N, dh=128]  = nodesT.T @ wv_slice (+bias via ones-row matmul)
  per head h: simT[j,i] = kT_h.T @ qT_h                  (j on partitions)
              PT = exp(simT * DH**-0.5)   (ScalarE, free scale, bf16 out)
              PT *= maskT                  (VectorE, bf16 2x mode)
              numT[0:64,i] / den[64,i] = [v_h | 1].T @ PT  (ones col -> denom)
              attnT_h = numT * recip(den)  (recip + DMA partition-broadcast)
  out[i,:] += attnT.T @ wo_slice           (contraction over both heads)
"""
import numpy as np
import ml_dtypes

import concourse.bass as bass
import concourse.bacc as bacc
import concourse.tile as tile
from concourse import mybir
from concourse.bass_utils import run_bass_kernel_spmd

B, N, DIM = 2, 2048, 512
H, DH = 8, 64
INNER = H * DH
SCALE = DH ** -0.5
NCORES = 8
HEADS_PER_CORE = 2
HG = 128            # inner columns per core (2 heads x 64)
NJB = N // 128      # 16 j-blocks
NISL = N // 512     # 4 i-slices of 512
NC_DIM = DIM // 128  # 4 contraction chunks over DIM

BF16 = mybir.dt.bfloat16
F32 = mybir.dt.float32
ts = bass.ts


def _build():
    nc = bacc.Bacc(monotonic_sem_count=0)
    nT_d = nc.declare_dram_parameter("nodesT", [DIM, N], BF16, isOutput=False)
    maskT_d = nc.declare_dram_parameter("maskT", [N, N], BF16, isOutput=False)
    wq_d = nc.declare_dram_parameter("wq_s", [DIM, HG], BF16, isOutput=False)
    wk_d = nc.declare_dram_parameter("wk_s", [DIM, HG], BF16, isOutput=False)
    wv_d = nc.declare_dram_parameter("wv_s", [DIM, HG], BF16, isOutput=False)
    wo_d = nc.declare_dram_parameter("wo_s", [HG, DIM], BF16, isOutput=False)
    bq_d = nc.declare_dram_parameter("bq_s", [HG, 1], F32, isOutput=False)
    bk_d = nc.declare_dram_parameter("bk_s", [HG, 1], F32, isOutput=False)
    bv_d = nc.declare_dram_parameter("bv_s", [1, HG], BF16, isOutput=False)
    out_d = nc.declare_dram_parameter("out", [N, DIM], F32, isOutput=True)

    with tile.TileContext(nc) as tc:
        with (
            tc.tile_pool(name="persist", bufs=1) as persist,
            tc.tile_pool(name="ptp", bufs=4) as ptp,
            tc.tile_pool(name="denp", bufs=2) as denp,
            tc.tile_pool(name="outp", bufs=3) as outp,
            # PSUM is 8 banks total; two 4-bank pools, one slot each, shared
            # across phases: psA = {q/k proj, sim, even o-proj}, psB = {v proj,
            # num, odd o-proj}.
            tc.tile_pool(name="psA", bufs=1, space="PSUM") as psA,
            tc.tile_pool(name="psB", bufs=1, space="PSUM") as psB,
        ):
            # ---- loads ----
            nT = persist.tile([128, NC_DIM, N], BF16)
            nc.sync.dma_start(
                out=nT[:], in_=nT_d.rearrange("(c p) n -> p c n", p=128)
            )
            wq = persist.tile([128, NC_DIM, HG], BF16)
            nc.sync.dma_start(
                out=wq[:], in_=wq_d.rearrange("(c p) m -> p c m", p=128)
            )
            wk = persist.tile([128, NC_DIM, HG], BF16)
            nc.sync.dma_start(
                out=wk[:], in_=wk_d.rearrange("(c p) m -> p c m", p=128)
            )
            wv = persist.tile([128, NC_DIM, HG], BF16)
            nc.sync.dma_start(
                out=wv[:], in_=wv_d.rearrange("(c p) m -> p c m", p=128)
            )
            wo = persist.tile([HG, DIM], BF16)
            nc.sync.dma_start(out=wo[:], in_=wo_d[:])
            bq = persist.tile([HG, 1], F32)
            nc.sync.dma_start(out=bq[:], in_=bq_d[:])
            bk = persist.tile([HG, 1], F32)
            nc.sync.dma_start(out=bk[:], in_=bk_d[:])
            bv = persist.tile([1, HG], BF16)
            nc.sync.dma_start(out=bv[:], in_=bv_d[:])
            ones = persist.tile([1, 128], BF16)
            nc.vector.memset(ones[:], 1.0)

            maskT = persist.tile([128, NJB, N], BF16)
            for jb in range(NJB):
                nc.sync.dma_start(
                    out=maskT[:, jb, :], in_=maskT_d[ts(jb, 128), :]
                )

            # ---- projections ----
            qT = persist.tile([128, N], BF16)
            kT = persist.tile([128, N], BF16)
            for dst, w, b in ((qT, wq, bq), (kT, wk, bk)):
                for half in range(2):
                    pps = psA.tile([128, N // 2], F32, tag=f"sim{half}")
                    for isl in range(2):
                        for c in range(NC_DIM):
                            nc.tensor.matmul(
                                pps[:, ts(isl, 512)],
                                lhsT=w[:, c, :],
                                rhs=nT[:, c, ts(half * 2 + isl, 512)],
                                start=(c == 0),
                                stop=(c == NC_DIM - 1),
                            )
                    nc.vector.tensor_scalar_add(
                        dst[:, ts(half, N // 2)], pps[:], b[:]
                    )

            # v rows [j, dh] with a ones column appended per head (so the
            # softmax denominator lands at partition 64 of the AV output):
            # cols 0:64 = head0 v, col 64 = 1, cols 65:129 = head1 v, col 129 = 1
            v_sb = persist.tile([128, NJB, 130], BF16)
            for jb in range(NJB):
                vps = psA.tile([128, HG], F32, tag=f"sim{jb % 2}")
                for c in range(NC_DIM):
                    nc.tensor.matmul(
                        vps[:],
                        lhsT=nT[:, c, ts(jb, 128)],
                        rhs=wv[:, c, :],
                        start=(c == 0),
                        stop=False,
                    )
                nc.tensor.matmul(
                    vps[:], lhsT=ones[:, 0:HG], rhs=bv[:], start=False, stop=True
                )
                nc.vector.tensor_copy(v_sb[:, jb, 0:64], vps[:, 0:64])
                nc.vector.tensor_copy(v_sb[:, jb, 65:129], vps[:, 64:128])
            nc.vector.memset(v_sb[:, :, 64:65], 1.0)
            nc.vector.memset(v_sb[:, :, 129:130], 1.0)

            # ---- attention per head ----
            attnT = persist.tile([128, N], BF16)
            for h in range(HEADS_PER_CORE):
                hp = ts(h, 64)  # partitions of this head's dh in qT/kT
                nps = psB.tile([65, N], F32, tag="B")
                for jb in range(NJB):
                    pt = ptp.tile([128, N], BF16, tag="pt")
                    # two 2-bank sim slots: ACT exps one half while PE
                    # fills the other -> ScalarE streams continuously
                    for half in range(2):
                        sps = psA.tile([128, N // 2], F32, tag=f"sim{half}")
                        for isl in range(2):
                            nc.tensor.matmul(
                                sps[:, ts(isl, 512)],
                                lhsT=kT[hp, ts(jb, 128)],
                                rhs=qT[hp, ts(half * 2 + isl, 512)],
                                start=True,
                                stop=True,
                            )
                        nc.scalar.activation(
                            out=pt[:, ts(half, N // 2)],
                            in_=sps[:],
                            func=mybir.ActivationFunctionType.Exp,
                            scale=SCALE,
                        )
                    nc.vector.tensor_mul(pt[:], pt[:], maskT[:, jb, :])
                    for isl in range(NISL):
                        nc.tensor.matmul(
                            nps[:, ts(isl, 512)],
                            lhsT=v_sb[:, jb, ts(h, 65)],
                            rhs=pt[:, ts(isl, 512)],
                            start=(jb == 0),
                            stop=(jb == NJB - 1),
                        )
                # attnT_h = numT * (1/den), den broadcast across partitions
                # copy the AV accumulator to SBUF right away so the PSUM
                # slot frees for the next head's accumulation; row 64 is the
                # softmax denominator (ones column in v_sb).
                nsb = denp.tile([65, N], F32, tag="nsb")
                nc.vector.tensor_copy(nsb[:], nps[:])
                # move the denominator row to partition 0 via DMA (cheap;
                # a 1-partition DVE copy would serialize over the free dim)
                den1 = denp.tile([1, N], F32, tag="den1")
                nc.sync.dma_start(out=den1[:], in_=nsb[64:65, :])
                rec1 = denp.tile([1, N], F32, tag="rec1")
                nc.vector.reciprocal_approx_fast(out=rec1[:], in_=den1[:])
                rec = denp.tile([64, N], F32, tag="rec")
                nc.gpsimd.partition_broadcast(rec[:], rec1[:])
                nc.vector.tensor_mul(attnT[hp, :], nsb[0:64, :], rec[:])

            # ---- output projection (contraction over both heads' inner dim) ----
            for ib in range(NJB):
                ops = psA.tile([128, DIM], F32, tag=f"sim{ib % 2}")
                nc.tensor.matmul(
                    ops[:], lhsT=attnT[:, ts(ib, 128)], rhs=wo[:],
                    start=True, stop=True,
                )
                osb = outp.tile([128, DIM], F32, tag="osb")
                nc.vector.tensor_copy(osb[:], ops[:])
                nc.sync.dma_start(out=out_d[ts(ib, 128), :], in_=osb[:])

    # Bacc.compile runs generate_event_semaphores, which splits multi-sem
    # waits down to the 1-wait-per-instruction limit this walrus enforces.
    nc.compile()

    # Bacc's dce_regs leaves the (unread) engine-preamble register writes
    # behind at this kernel size, with deferred reg_id=-1 — walrus then
    # fails "Reg has not been allocated yet".  Nothing reads them, so any
    # valid unique per-engine id works.
    from collections import defaultdict

    next_id = defaultdict(lambda: 8)
    for a in nc.m.functions[0].allocations:
        if type(a).__name__ == "Register" and a.reg_id == -1:
            a.reg_id = next_id[str(a.engine)]
            next_id[str(a.engine)] += 1
    return nc


_NC_CACHE = None


def _get_nc():
    global _NC_CACHE
    if _NC_CACHE is None:
        _NC_CACHE = _build()
    return _NC_CACHE


def _prep_in_maps(nodes, edge_mask, wq, bq, wkv, bkv, wo, bo):
    bf16 = ml_dtypes.bfloat16
    wk_full, wv_full = wkv[:, :INNER], wkv[:, INNER:]
    bk_full, bv_full = bkv[:INNER], bkv[INNER:]
    per_batch = []
    for b in range(B):
        per_batch.append(
            (
                np.ascontiguousarray(nodes[b].T).astype(bf16),
                np.ascontiguousarray(edge_mask[b].T).astype(bf16),
            )
        )
    in_maps = []
    for core in range(NCORES):
        b, g = core // 4, core % 4
        cs = slice(g * HG, (g + 1) * HG)
        nT_b, maskT_b = per_batch[b]
        in_maps.append(
            {
                "nodesT": nT_b,
                "maskT": maskT_b,
                "wq_s": np.ascontiguousarray(wq[:, cs]).astype(bf16),
                "wk_s": np.ascontiguousarray(wk_full[:, cs]).astype(bf16),
                "wv_s": np.ascontiguousarray(wv_full[:, cs]).astype(bf16),
                "wo_s": np.ascontiguousarray(wo[cs, :]).astype(bf16),
                "bq_s": np.ascontiguousarray(bq[cs]).reshape(HG, 1).astype(np.float32),
                "bk_s": np.ascontiguousarray(bk_full[cs]).reshape(HG, 1).astype(np.float32),
                "bv_s": np.ascontiguousarray(bv_full[cs]).reshape(1, HG).astype(bf16),
            }
        )
    return in_maps


def kernel(nodes, edge_mask, wq, bq, wkv, bkv, wo, bo, _trace=False, _trace_kwargs=None):
    nodes = np.asarray(nodes, dtype=np.float32)
    edge_mask = np.asarray(edge_mask)
    wq = np.asarray(wq, dtype=np.float32)
    bq = np.asarray(bq, dtype=np.float32)
    wkv = np.asarray(wkv, dtype=np.float32)
    bkv = np.asarray(bkv, dtype=np.float32)
    wo = np.asarray(wo, dtype=np.float32)
    bo = np.asarray(bo, dtype=np.float32)

    nc = _get_nc()
    in_maps = _prep_in_maps(nodes, edge_mask, wq, bq, wkv, bkv, wo, bo)
    kw = {}
    if _trace:
        kw = dict(trace=True, **(_trace_kwargs or {}))
    res = run_bass_kernel_spmd(nc, in_maps, list(range(NCORES)), **kw)
    out = np.zeros((B, N, DIM), np.float32)
    for core in range(NCORES):
        out[core // 4] += res.results[core]["out"]
    out += bo[None, None, :]
    if _trace:
        return out, res
    return out
